# revision 57
# baseline (speedup 1.0000x reference)
"""Bahdanau attention Trainium2 kernel.

Problem: B=8, T=256, S=256, H=512 (fp32 I/O).
  Ws_q = q @ W_s.T ; Wh_e = e @ W_h.T
  energies[b,t,s] = v . tanh(Ws_q[b,t,:] + Wh_e[b,s,:])   (masked s >= len_b)
  attn = softmax_s(energies); ctx = attn @ e
  out = tanh(concat([ctx, q]) @ W_out.T)

Sharding: sequence-parallel over T — core c handles t in [c*32, (c+1)*32)
for ALL batches, balancing src_lengths sparsity across cores.

Per-core dataflow (bf16 compute, fp32 accumulation):
  PE   : Ws_q^T [o,t] and Wh_e^T [o,s] projections (o on partitions)
  DVE  : X[o, t, s] = es[o,s] + qs[o,t] via one BROADCAST tensor_tensor
         per (b, oc): es with stride-0 over t, qs pre-duplicated into
         pairs (qs2[o,t,2]) so the qs operand's innermost AP dim is
         [1,2] — every operand stays 2-byte/step-1, keeping the DVE in
         2x_1P mode (~0.54 ns/free-elem vs 0.83 for per-t tensor_scalar,
         measured; the tensor_scalar path also needs 32x more instrs).
  ACT  : tanh(X) in place, one instr per (b, oc), F = 32*L. ACT runs
         ACTIVATE at 1 elem/cycle/lane @1.2GHz + ~290ns/instr — this is
         the bottleneck engine (~87% busy): ~155us of pure tanh stream.
  PE   : energies[t,s] = sum_o v_o X[o,t,s] — M=1 matmuls col-tiled
         4-wide via tile_position, n = Gt*L <= 512 f32 (Gt in {2,4}
         targets per matmul, halving/quartering the vdot instr count)
  DMA  : gather PSUM rows {0,32,64,96} x Gt slots -> energies [32t, s]
         (partition->partition, out partition stride Gt)
  DVE/ACT: masked softmax (exp over the first len_b cols + zeroed weight
         tail, with accum_out for the row sums)
  DMA  : xbar-transpose of weights [32,s] -> [s,32]
  PE   : ctx^T[h,t] = enc^T @ w^T ; out[t,o] = tanh(comb^T.T @ W_out^T)

Schedule: batches in descending-L order (s-extents padded to even
length only). Fill: wht/wst on the SP HWDGE queue while qt/encT(b0) go
on the ACT queue (idle then), and batch 0
round-robins es-proj(oc)/qs-proj(oc)/adds(oc) so the first tanh starts
~8us in. Steady state: tail(b-1) segments are emitted interleaved
between head(b)'s per-oc adds (vdot after oc1, softmax after oc2,
ctx/out after oc3) so in-order engine queues reach tail work early; ACT
then runs gap-free. Drain: the last (smallest-L) batch's tail is split
into two t-halves through softmax, with a single combined ctx/out
whose gate-free qt-side accumulation matmuls are hoisted ahead of the
terminal softmax chain (PSUM group split across emission is legal as
long as no other group touches the same bank in between).

Measured rates (clean R-sweep microbenches, this container): DVE TT
bf16 2x_1P ~0.52 ns/free-elem + ~80ns/instr with NO long-op drain
penalty; DVE tensor_scalar 4x ~0.26 ns/elem but ~200ns/instr in
context; ACT ACTIVATE exactly (F + 352 cyc)/1.2GHz at ANY F (the 2x
bf16 rate suggested by early loop-confounded benches is NOT real).
Rejected: fused bias-tanh on ACT (per-t bias forces F=L, fixed cost
dominates); GPSIMD offload (shares the DVE SBUF port, ~2x slower,
us-scale sync); polynomial tanh on DVE (saturation needs the LUT —
odd polys hit 0.1+ abs error or dive past the fit range).

HW notes: PSUM accumulation groups must not interleave within a
(partition, bank) zero-region; DMA cannot read PSUM; single-DMA
free-dim->partition scatter silently misplaces data; energ-style
[4, 256] partition-gather DMA ~500ns, [32,128] xbar transpose ~1.3us.
"""

import functools

import ml_dtypes
import numpy as np

B, T, S, H = 8, 256, 256, 512
NCORES = 8
TC = T // NCORES  # 32 target positions per core
KC = H // 128     # 4 contraction chunks
OC = H // 128     # 4 output-feature chunks

_BF16 = ml_dtypes.bfloat16


def _ceil4(x: int) -> int:
    # pad to even (pairs trick needs L%2==0); min 4 for tiny-AP safety
    return max(4, (x + 1) // 2 * 2)


@functools.lru_cache(maxsize=8)
def _build(lens: tuple, loop_n: int | None = None, stages: int = 3):
    """Build + compile the per-core Bass program with per-batch s-extents
    baked in. Same program runs on all 8 cores (inputs differ)."""
    import concourse.mybir as mybir
    import concourse.tile as tile
    from concourse import bacc

    f32 = mybir.dt.float32
    bf16 = mybir.dt.bfloat16
    AF = mybir.ActivationFunctionType

    Ls = [_ceil4(l) for l in lens]

    nc = bacc.Bacc("TRN2", target_bir_lowering=False, debug=False)

    # All inputs are host-pre-arranged into SBUF layout [128, free].
    qt_d = nc.dram_tensor("qt", [128, KC, B, TC], bf16, kind="ExternalInput")
    encT_d = nc.dram_tensor("encT", [B, 128, KC, S], bf16, kind="ExternalInput")
    enc_d = nc.dram_tensor("enc", [B, 128, S // 128, H], bf16, kind="ExternalInput")
    wst_d = nc.dram_tensor("wst", [128, KC, H], bf16, kind="ExternalInput")
    wht_d = nc.dram_tensor("wht", [128, KC, H], bf16, kind="ExternalInput")
    v_d = nc.dram_tensor("v", [128, KC], bf16, kind="ExternalInput")
    wot_d = nc.dram_tensor("wot", [128, 2 * KC, H], bf16, kind="ExternalInput")
    out_d = nc.dram_tensor("out", [B, TC, H], f32, kind="ExternalOutput")

    import contextlib

    with tile.TileContext(nc) as tc:
        loop_cm = (
            tc.For_i(
                0, loop_n, 1,
                hint_engines=(
                    mybir.EngineType.PE, mybir.EngineType.DVE,
                    mybir.EngineType.Activation, mybir.EngineType.SP,
                ),
            )
            if loop_n is not None
            else contextlib.nullcontext()
        )
        with (
            tc.tile_pool(name="const", bufs=1) as constp,
            tc.tile_pool(name="enc", bufs=3) as encp,
            tc.tile_pool(name="es", bufs=2) as esp,
            tc.tile_pool(name="x", bufs=2) as xp,
            tc.tile_pool(name="xfill", bufs=1) as xfp,
            tc.tile_pool(name="sm", bufs=3) as smp,
            tc.tile_pool(name="outs", bufs=3) as outp,
            tc.tile_pool(name="psA", bufs=3, space="PSUM") as psA,
            tc.tile_pool(name="psV", bufs=2, space="PSUM") as psV,
            tc.tile_pool(name="psC", bufs=1, space="PSUM") as psC,
            tc.tile_pool(name="psO", bufs=1, space="PSUM") as psO,
            loop_cm,
        ):
            # ---- persistent weights/activations ----
            # Fill-critical DMAs split across both HWDGE queues: wht + batch
            # 0's encT on the ACT queue (idle during fill; LoadActFuncSet
            # queues behind them, still long before the first tanh), qt +
            # wst on the SP queue. Steady-state DMAs all go on SP so issue
            # costs never eat into the tanh-bound ACT budget.
            wht = constp.tile([128, KC, H], bf16)
            nc.sync.dma_start(wht[:], wht_d[:])
            qt_sb = constp.tile([128, KC, B, TC], bf16)
            nc.scalar.dma_start(qt_sb[:], qt_d[:])
            wst = constp.tile([128, KC, H], bf16)
            nc.sync.dma_start(wst[:], wst_d[:])
            # v/wot are not needed until the first tail; their DMA issue is
            # deferred below so batch 0's encT/enc aren't queued behind them.
            v_sb = constp.tile([128, KC], bf16)
            wot = constp.tile([128, 2 * KC, H], bf16)

            # ---- Ws_q^T for all (b, t), duplicated into pairs:
            # qs2[o-part, oc, b, t, j] = Ws_q[o, b, t] for j in {0,1}.
            # The pair duplication keeps the broadcast tensor_tensor's qs
            # operand innermost AP at [1, 2] (2-byte step-1) => 2x_1P mode.
            # Per-oc tiles: a single qs2 tile would make the first adds
            # falsely wait on all 4 evacs (tile-granular dep tracking).
            qs2 = []

            def emit_qsproj_oc(oc):
                q2 = constp.tile([128, B, TC, 2], bf16, tag=f"qs2_{oc}",
                                 name=f"qs2_{oc}")
                ps = psA.tile([128, B * TC], f32, tag="proj")
                for kc in range(KC):
                    nc.tensor.matmul(
                        ps[:],
                        wst[:, kc, oc * 128 : (oc + 1) * 128],
                        qt_sb[:, kc, :, :],
                        start=(kc == 0),
                        stop=(kc == KC - 1),
                    )
                nc.vector.tensor_copy(
                    q2[:].rearrange("p b t two -> p (b t) two"),
                    ps[:].unsqueeze(2).broadcast_to([128, B * TC, 2]),
                )
                qs2.append(q2)

            # Software-pipelined emission: engines execute their streams in
            # order; tail(b-1) is emitted in segments interleaved between
            # head(b)'s per-oc add groups.
            state = {}
            aux = {}

            def head_dma(b, first=False):
                # load encoder (both layouts), full-S tiles for clean DMA
                encT_b = encp.tile([128, KC, S], bf16, tag="encT")
                (nc.scalar if first else nc.sync).dma_start(encT_b[:], encT_d[b])
                enc_b = encp.tile([128, S // 128, H], bf16, tag="enc")
                nc.sync.dma_start(enc_b[:], enc_d[b])
                state[b] = ([None] * OC, enc_b, [None] * OC, encT_b)

            def head_esproj(b, oc):
                # Wh_e^T: es[o-part, s] for one oc
                L = Ls[b]
                _, _, es, encT_b = state[b]
                ps = psA.tile([128, L], f32, tag="proj")
                for kc in range(KC):
                    nc.tensor.matmul(
                        ps[:],
                        wht[:, kc, oc * 128 : (oc + 1) * 128],
                        encT_b[:, kc, :L],
                        start=(kc == 0),
                        stop=(kc == KC - 1),
                    )
                e = esp.tile([128, L], bf16, tag=f"es{oc}")
                nc.vector.tensor_copy(e[:], ps[:])
                es[oc] = e

            def head_adds(b, oc, fill_split=False, vhalf=None, merged=False):
                """Broadcast-TT adds + one tanh for one oc of batch b.
                fill_split: two half-t tiles from a dedicated pool with a
                tanh each, so the first tanh starts after half the adds
                (only worth it for the very first adds of the program)."""
                L = Ls[b]
                X, enc_b, es, _ = state[b]
                if fill_split:
                    HT = TC // 2
                    halves = []
                    for h2 in range(2):
                        xh = xfp.tile([128, HT, L], bf16, tag=f"xf{h2}",
                                      name=f"xf{h2}")
                        in0 = (
                            es[oc][:]
                            .unsqueeze(1)
                            .broadcast_to([128, HT, L])
                            .rearrange("p g (h two) -> p g h two", two=2)
                        )
                        in1 = (
                            qs2[oc][:, b, h2 * HT : (h2 + 1) * HT, :]
                            .unsqueeze(2)
                            .broadcast_to([128, HT, L // 2, 2])
                        )
                        out = xh[:].rearrange("p g (h two) -> p g h two", two=2)
                        nc.vector.tensor_tensor(out, in0, in1, mybir.AluOpType.add)
                        if stages != 4:
                            nc.scalar.activation(xh[:], xh[:], AF.Tanh)
                        halves.append(xh)
                    X[oc] = halves
                    return
                if vhalf is not None:
                    # virtual half-batch: t-range [vhalf*16, vhalf*16+16).
                    # half a reuses the x-ring (slot freed 2 batches ago);
                    # half b gets dedicated xfill-pool tiles so its adds
                    # don't WAR-wait on vdot(prev) reading the ring slot.
                    HT2 = TC // 2
                    tb = vhalf * HT2
                    if vhalf == 0:
                        xh = xp.tile([128, HT2, L], bf16, tag=f"x{oc}",
                                     name=f"xv{oc}{vhalf}")
                    else:
                        vtag = f"xf{oc}" if oc < 2 else f"xv{oc}"
                        xh = xfp.tile([128, HT2, L], bf16, tag=vtag,
                                      name=f"xv{oc}{vhalf}")
                    in0 = (
                        es[oc][:]
                        .unsqueeze(1)
                        .broadcast_to([128, HT2, L])
                        .rearrange("p g (h two) -> p g h two", two=2)
                    )
                    in1 = (
                        qs2[oc][:, b, tb : tb + HT2, :]
                        .unsqueeze(2)
                        .broadcast_to([128, HT2, L // 2, 2])
                    )
                    out = xh[:].rearrange("p g (h two) -> p g h two", two=2)
                    nc.vector.tensor_tensor(out, in0, in1, mybir.AluOpType.add)
                    if stages != 4:
                        nc.scalar.activation(xh[:], xh[:], AF.Tanh)
                    if vhalf == 0:
                        X[oc] = [xh, None]
                    else:
                        X[oc][1] = xh
                    return
                if merged:
                    # last batch: all 4 oc in ONE tile -> one tanh (F=4*32*L)
                    # saves 3 ACT fixed costs and starts the drain's vdot
                    # earlier. Rides tag x0 (its ring slot is 2 batches old).
                    if oc == 0:
                        aux[(b, "xall")] = xp.tile(
                            [128, OC, TC, L], bf16, tag="x0", name="xall"
                        )
                    x_all = aux[(b, "xall")]
                    in0 = (
                        es[oc][:]
                        .unsqueeze(1)
                        .broadcast_to([128, TC, L])
                        .rearrange("p g (h two) -> p g h two", two=2)
                    )
                    in1 = (
                        qs2[oc][:, b, :, :]
                        .unsqueeze(2)
                        .broadcast_to([128, TC, L // 2, 2])
                    )
                    out = x_all[:, oc].rearrange(
                        "p g (h two) -> p g h two", two=2
                    )
                    nc.vector.tensor_tensor(out, in0, in1, mybir.AluOpType.add)
                    X[oc] = x_all[:, oc]
                    if oc == OC - 1 and stages != 4:
                        nc.scalar.activation(x_all[:], x_all[:], AF.Tanh)
                    return
                G = TC
                x = xp.tile([128, TC, L], bf16, tag=f"x{oc}", name=f"x{oc}")
                for t0 in range(0, TC, G):
                    in0 = (
                        es[oc][:]
                        .unsqueeze(1)
                        .broadcast_to([128, G, L])
                        .rearrange("p g (h two) -> p g h two", two=2)
                    )
                    in1 = (
                        qs2[oc][:, b, t0 : t0 + G, :]
                        .unsqueeze(2)
                        .broadcast_to([128, G, L // 2, 2])
                    )
                    out = x[:, t0 : t0 + G, :].rearrange(
                        "p g (h two) -> p g h two", two=2
                    )
                    nc.vector.tensor_tensor(out, in0, in1, mybir.AluOpType.add)
                if stages != 4:
                    nc.scalar.activation(x[:], x[:], AF.Tanh)
                X[oc] = x

            def xslice(b, t):
                """X[o, t, :] SBUF slice for target t of batch b, per oc."""
                X, _, _, _ = state[b]
                return [X[oc][:, t, :] for oc in range(OC)]

            def tail_vdot(b, t0=0, nt=TC, part=""):
                L = Ls[b]
                if stages == 1:
                    return
                # energies[t, s] = sum_o v_o X[o, t, s]: M=1 matmuls, 8 t's
                # per PSUM tile (4 col groups x 2 bank slots), wide evac,
                # partition->partition DMA gather. Accumulation groups
                # sharing a (partition, bank) zero-region must not
                # interleave; col groups (distinct partitions) may.
                energ = smp.tile([nt, L], f32, tag=f"energ{part}")
                X = state[b][0]
                # Gt targets per M=1 matmul (n = Gt*L <= 512 f32, one PSUM
                # bank row): halves/quarters the v-dot instruction count.
                Gt = 4 if L <= 128 else 2
                TPT = 4 * Gt  # t's per psq tile (4 col groups)
                for h in range(nt // TPT):
                    psq = psV.tile([128, Gt, 512 // Gt], f32, tag="vdot")
                    for j in range(4):
                        tj = t0 + h * TPT + j * Gt
                        for oc in range(OC):
                            xoc = X[oc]
                            if isinstance(xoc, list):  # fill-split halves
                                HT2 = TC // 2
                                rhs = xoc[tj // HT2][:, tj % HT2 : tj % HT2 + Gt, :]
                            else:
                                rhs = xoc[:, tj : tj + Gt, :]
                            nc.tensor.matmul(
                                psq[32 * j : 32 * j + 1, :, :L],
                                v_sb[:, oc : oc + 1],
                                rhs,
                                start=(oc == 0),
                                stop=(oc == OC - 1),
                                tile_position=(0, 32 * j),
                            )
                    vscr = smp.tile([128, Gt, L], f32, tag="vscr")
                    nc.vector.tensor_copy(vscr[:], psq[:, :, :L])
                    vsr = vscr.rearrange("(g r) n f -> g r n f", r=32)
                    ev = energ.rearrange(
                        "(hh j g) f -> hh j g f", j=4, g=Gt
                    )
                    for n in range(Gt):
                        nc.sync.dma_start(ev[h, :, n, :], vsr[:, 0, n, :])
                aux[(b, part)] = {"energ": energ}

            def tail_softmax(b, t0=0, nt=TC, part=""):
                L = Ls[b]
                ln = min(int(lens[b]), S)
                SC = (L + 127) // 128
                L128 = SC * 128
                if stages < 2:
                    return
                energ = aux[(b, part)]["energ"]
                if stages == 2:
                    ob = outp.tile([32, 16], f32, tag="ob1")
                    nc.vector.tensor_copy(ob[:], energ[:, :16])
                    nc.sync.dma_start(out_d[b][:, :16], ob[:])
                    return
                # softmax over s (energies bounded by sum|v| ~ 20: raw exp
                # is safe — skip max-subtract). The s >= len_b mask is
                # applied by exp-ing only the first ln columns and zeroing
                # the weight tail.
                w_sb = smp.tile([nt, L128], bf16, tag=f"w{part}")
                if L128 > ln:
                    nc.vector.memset(w_sb[:, ln:], 0.0)
                sm = smp.tile([nt, 1], f32, tag=f"sm{part}")
                nc.scalar.activation(
                    w_sb[:, :ln], energ[:, :ln], AF.Exp, accum_out=sm[:]
                )
                rs = smp.tile([nt, 1], f32, tag=f"rs{part}")
                nc.vector.reciprocal(rs[:], sm[:])
                nc.vector.tensor_scalar_mul(w_sb[:, :ln], w_sb[:, :ln], rs[:])

                # w^T via DMA xbar transpose: [nt, L128] -> [L128, nt]
                wT = smp.tile([128, SC, nt], bf16, tag=f"wT{part}")
                for sc in range(SC):
                    nc.sync.dma_start_transpose(
                        wT[:, sc, :], w_sb[:, sc * 128 : (sc + 1) * 128]
                    )
                aux[(b, part)]["wT"] = wT

            def tail_out(b, t0=0, nt=TC, part=""):
                L = Ls[b]
                SC = (L + 127) // 128
                if stages < 3:
                    return
                enc_b = state[b][1]
                wT = aux[(b, part)]["wT"]
                # ctx^T[h, t] = sum_s enc[s, h] * w[t, s]; all 4 oc slots in
                # one PSUM bank-row (groups are sequential per slot — legal)
                psc_full = psC.tile([128, OC, TC], f32, tag="ctx")
                psc = psc_full[:, :, :nt]
                for oc in range(OC):
                    for sc in range(SC):
                        nc.tensor.matmul(
                            psc[:, oc, :],
                            enc_b[:, sc, oc * 128 : (oc + 1) * 128],
                            wT[:, sc, :],
                            start=(sc == 0),
                            stop=(sc == SC - 1),
                        )
                ctxT_full = outp.tile([128, OC, TC], bf16, tag="ctxT")
                ctxT = ctxT_full[:, :, :nt]
                nc.vector.tensor_copy(ctxT, psc[:])

                # out[t, o] = tanh(sum_k comb^T[k, t] * W_out[o, k])
                pso_full = psO.tile([32, H], f32, tag="outp")
                pso = pso_full[:nt, :]
                for kc in range(2 * KC):
                    lhsT = (
                        ctxT[:, kc, :]
                        if kc < OC
                        else qt_sb[:, kc - OC, b, t0 : t0 + nt]
                    )
                    nc.tensor.matmul(
                        pso,
                        lhsT,
                        wot[:, kc, :],
                        start=(kc == 0),
                        stop=(kc == 2 * KC - 1),
                    )
                ob_full = outp.tile([32, H], f32, tag="ob")
                ob = ob_full[:nt, :]
                nc.scalar.activation(ob, pso, AF.Tanh)
                nc.sync.dma_start(out_d[b][t0 : t0 + nt], ob)
                del aux[(b, part)]
                if t0 + nt == TC:
                    del state[b]

            def tail_ctx(b, t0, nt, part):
                # ctx matmuls for one t-half into a batch-shared psc tile.
                L = Ls[b]
                SC = (L + 127) // 128
                enc_b = state[b][1]
                wT = aux[(b, part)]["wT"]
                if "psc" not in aux[(b, "a")]:
                    aux[(b, "a")]["psc"] = psC.tile(
                        [128, OC, TC], f32, tag="ctx", name="psc_shared"
                    )
                psc = aux[(b, "a")]["psc"]
                for oc in range(OC):
                    for sc in range(SC):
                        nc.tensor.matmul(
                            psc[:, oc, t0 : t0 + nt],
                            enc_b[:, sc, oc * 128 : (oc + 1) * 128],
                            wT[:, sc, :],
                            start=(sc == 0),
                            stop=(sc == SC - 1),
                        )

            def tail_final_qt(b):
                # qt-side half of the final out-proj accumulation: gated on
                # nothing but qt/wot, so PE runs it while softmax-b drains.
                pso = psO.tile([32, H], f32, tag="outp")
                for kc in range(KC):
                    nc.tensor.matmul(
                        pso[:],
                        qt_sb[:, kc, b, :],
                        wot[:, KC + kc, :],
                        start=(kc == 0),
                        stop=False,
                    )
                aux[(b, "a")]["pso"] = pso

            def tail_final(b):
                # ctx-side accumulation + tanh + DMA for all TC t's
                psc = aux[(b, "a")]["psc"]
                pso = aux[(b, "a")]["pso"]
                ctxT = outp.tile([128, OC, TC], bf16, tag="ctxT")
                nc.vector.tensor_copy(ctxT[:], psc[:])
                for kc in range(KC):
                    nc.tensor.matmul(
                        pso[:],
                        ctxT[:, kc, :],
                        wot[:, kc, :],
                        start=False,
                        stop=(kc == KC - 1),
                    )
                ob = outp.tile([32, H], f32, tag="ob")
                nc.scalar.activation(ob[:], pso[:], AF.Tanh)
                nc.sync.dma_start(out_d[b], ob[:])
                del aux[(b, "a")]
                del aux[(b, "b")]
                del state[b]

            # Descending-L order: the pipeline tail drain (last batch's
            # tail with no head to overlap) is paid on the smallest batch.
            order = sorted(range(B), key=lambda b: -Ls[b])
            # Batch 0 fill: per-oc round-robin of es-proj / qs-proj / adds
            # so the first tanh starts as soon as es[0]+qs2[0] exist,
            # instead of after all 8 projections and evacs.
            b0 = order[0]
            head_dma(b0, first=True)
            for oc in range(OC):
                head_esproj(b0, oc)
                emit_qsproj_oc(oc)
                if oc == OC - 1:
                    nc.sync.dma_start(v_sb[:], v_d[:])
                    nc.sync.dma_start(wot[:], wot_d[:])
                head_adds(b0, oc, fill_split=(oc == 0))
            prev = b0
            for b in order[1:]:
                head_dma(b)
                for oc in range(OC):
                    head_esproj(b, oc)
                if b != order[-1]:
                    head_adds(b, 0)
                    head_adds(b, 1)
                    tail_vdot(prev)
                    head_adds(b, 2)
                    tail_softmax(prev)
                    head_adds(b, 3)
                    tail_out(prev)
                else:
                    # Last batch runs as TWO virtual half-batches (t 0-15,
                    # 16-31) through the whole pipeline: half a's tail hides
                    # inside half b's head window, so the true drain is only
                    # half b's tail (~half the work). The x-pool ring
                    # provides exactly the two buffers needed per oc.
                    head_adds(b, 0, merged=True)
                    head_adds(b, 1, merged=True)
                    tail_vdot(prev)
                    head_adds(b, 2, merged=True)
                    tail_softmax(prev)
                    head_adds(b, 3, merged=True)
                    HT = TC // 2
                    tail_vdot(b, 0, HT, "a")
                    tail_out(prev)
                    tail_softmax(b, 0, HT, "a")
                    tail_vdot(b, HT, HT, "b")
                    tail_ctx(b, 0, HT, "a")
                    tail_final_qt(b)
                    tail_softmax(b, HT, HT, "b")
                    tail_ctx(b, HT, HT, "b")
                    tail_final(b)
                prev = b

    nc.compile()
    return nc


def _prep_inputs(query, encoder_outputs, src_lengths, W_s, W_h, v, W_out):
    """Host-side: cast to bf16 and pre-arrange into SBUF layouts."""
    q = np.asarray(query, np.float32)
    e = np.asarray(encoder_outputs, np.float32)

    # [128, KC, B, TC] per core: qt[p, kc, b, t] = q[b, c*TC+t, kc*128+p]
    # build once for full T then slice per core.
    qt_full = np.transpose(
        q.reshape(B, T, KC, 128), (2, 3, 0, 1)
    )  # [KC, 128, B, T]
    qt_full = np.ascontiguousarray(np.swapaxes(qt_full, 0, 1)).astype(_BF16)
    # -> [128, KC, B, T]

    # encT[b, p, kc, s] = e[b, s, kc*128+p]
    encT = np.ascontiguousarray(
        np.transpose(e.reshape(B, S, KC, 128), (0, 3, 2, 1))
    ).astype(_BF16)
    # enc[b, p, sc, h] = e[b, sc*128+p, h]
    enc = np.ascontiguousarray(
        np.transpose(e.reshape(B, S // 128, 128, H), (0, 2, 1, 3))
    ).astype(_BF16)

    # wst[p, kc, o] = W_s[o, kc*128+p]
    wst = np.ascontiguousarray(
        np.transpose(np.asarray(W_s, np.float32).reshape(H, KC, 128), (2, 1, 0))
    ).astype(_BF16)
    wht = np.ascontiguousarray(
        np.transpose(np.asarray(W_h, np.float32).reshape(H, KC, 128), (2, 1, 0))
    ).astype(_BF16)
    # v[p, kc] = v[kc*128+p]
    v_pre = np.ascontiguousarray(
        np.asarray(v, np.float32).reshape(KC, 128).T
    ).astype(_BF16)
    # wot[p, kc, o] = W_out[o, kc*128+p]   (k = 2H contraction)
    wot = np.ascontiguousarray(
        np.transpose(np.asarray(W_out, np.float32).reshape(H, 2 * KC, 128), (2, 1, 0))
    ).astype(_BF16)

    lens = tuple(int(x) for x in np.asarray(src_lengths).reshape(-1))
    return qt_full, encT, enc, wst, wht, v_pre, wot, lens


def kernel(query, encoder_outputs, src_lengths, W_s, W_h, v, W_out):
    from concourse import bass_utils

    qt_full, encT, enc, wst, wht, v_pre, wot, lens = _prep_inputs(
        query, encoder_outputs, src_lengths, W_s, W_h, v, W_out
    )
    nc = _build(lens)

    in_maps = []
    for c in range(NCORES):
        qt_c = np.ascontiguousarray(
            qt_full[:, :, :, c * TC : (c + 1) * TC]
        )
        in_maps.append(
            {
                "qt": qt_c,
                "encT": encT,
                "enc": enc,
                "wst": wst,
                "wht": wht,
                "v": v_pre,
                "wot": wot,
            }
        )

    res = bass_utils.run_bass_kernel_spmd(nc, in_maps, core_ids=list(range(NCORES)))

    out = np.empty((B, T, H), np.float32)
    for c in range(NCORES):
        out[:, c * TC : (c + 1) * TC, :] = res.results[c]["out"]
    return out
